# revision 16
# baseline (speedup 1.0000x reference)
"""Trainium2 Bass kernel: dual cross-attention block (nn_CA_36670430773307).

Full-input contract: kernel(**inputs) takes the complete unsharded tensors and
returns the complete (4, 4096, 512) output.

Sharding: 8 cores = batch(4) x direction(2). Each core computes one full
cross-attention direction (t->i or i->t) for one batch element.

v3 design (HW-measured microbenchmarks behind every choice):
  - sim (q@kT): bf16, K=64 head pairs row-grouped at partitions 0-63/64-127;
    a pair issues every ~260ns (true PE row-group concurrency).
  - exp: split Scalar (native Exp, PSUM->fp16, ~1.07ns/col) and DVE
    (Schraudolph-to-fp16 bits: int16 = round(1024/ln2*x + 15*1024-44),
    bit-reinterpreted as fp16; round-to-nearest conversion confirmed).
    GPSIMD cannot read PSUM so it only triggers DMAs.
  - av (attn@v): fp16, K=128 per j-tile. vaug [128, 128] per (jt, head):
    even heads: cols 0-63 = v, 64-127 = ones; odd heads flipped. The av
    output then carries the softmax denominator replicated on the OPPOSITE
    64 partitions from the numerator, and numerators of a head pair land on
    complementary partition halves -> out-projection runs K=128.
  - softmax 1/D: Ln then Exp(-x+ln16) on Scalar (4e-5 rel err), computed on
    the D rows in place, one SBUF->SBUF DMA to shift to the numerator's
    partitions, one DVE multiply (PSUM x SBUF -> fp16 aout).
  - out projection: fp16, K=128 (head pair per matmul), x16 descale folded
    into the final PSUM->SBUF copy.
  - gamma/beta: gamma folded into the projection weights host-side (exact);
    beta is guaranteed zero by the problem spec (asserted).
Precision (numpy pipeline sim): rel err ~1.0e-2 vs the 2e-2 gate.
"""

import numpy as np
import ml_dtypes

import concourse.bass as bass
import concourse.mybir as mybir
import concourse.tile as tile
from concourse.bass_utils import run_bass_kernel_spmd
from concourse.masks import make_identity

N = 2048            # tokens per stream
D = 512             # model dim
H = 8               # heads
HD = 64             # head dim
P = 128             # SBUF partitions
NT = N // P         # 16 token tiles
DC = D // P         # 4 model-dim chunks
IC = 512            # i-chunk (PSUM bank free size fp32)
LN_EPS = 1e-5

F32 = mybir.dt.float32
BF16 = mybir.dt.bfloat16
FP16 = mybir.dt.float16
I16 = mybir.dt.int16
ALU = mybir.AluOpType
ACTF = mybir.ActivationFunctionType

S16_A = 1024.0 / np.log(2.0)   # schraudolph-to-fp16: bits = A*x + B
S16_B = 15.0 * 1024 - 44.0
REC_SCALE = 16.0               # folded into Exp(-lnD + ln REC_SCALE)
OUT_DESCALE = 1.0 / REC_SCALE

LAST_EXEC_NS = None
_NC_CACHE = None


def _legalize_waits(js):
    """Walrus encodes ONE sync wait per instruction; split extras onto
    EventSemaphore instructions on the same engine."""
    for f in js["functions"]:
        for b in f["blocks"]:
            out = []
            for ins in b["instructions"]:
                si = ins.get("sync_info") or {}
                ow = si.get("on_wait") or []
                if len(ow) > 1:
                    for k, w in enumerate(ow[:-1]):
                        out.append({
                            "debug": ins.get("debug"),
                            "engine": ins["engine"],
                            "ins": [], "outs": [],
                            "name": f"{ins['name']}_w{k}",
                            "opcode": "EventSemaphore",
                            "sync_info": {"on_update": [], "on_wait": [w]},
                        })
                    si = dict(si)
                    si["on_wait"] = [ow[-1]]
                    ins = dict(ins)
                    ins["sync_info"] = si
                out.append(ins)
            b["instructions"] = out


def _build_program():
    nc = bass.Bass()

    xq = nc.declare_dram_parameter("xq", [N, D], F32, isOutput=False)
    xkv = nc.declare_dram_parameter("xkv", [N, D], F32, isOutput=False)
    wq = nc.declare_dram_parameter("wq", [D, D], BF16, isOutput=False)
    wkv = nc.declare_dram_parameter("wkv", [D, 2 * D], BF16, isOutput=False)
    wout16 = nc.declare_dram_parameter("wout16", [P, H // 2, D], FP16,
                                       isOutput=False)
    outs = [
        nc.declare_dram_parameter(f"out{g}", [P, 4, D], F32, isOutput=True)
        for g in range(NT // 4)
    ]

    with tile.TileContext(nc) as tc:
        _body(tc, xq, xkv, wq, wkv, wout16, outs)

    import json
    js = json.loads(nc.to_json_bytes())
    _legalize_waits(js)
    legalized = json.dumps(js).encode()
    nc.to_json_bytes = lambda: legalized
    return nc


def _phase_a(tc, lnx, lns, lnxs, ps_pool, src, xT, ident, eps_sb):
    """LayerNorm one stream token-major (gamma/beta folded into weights
    host-side), PE-transpose into d-major xT."""
    nc = tc.nc
    xbig = lnx.tile([P, NT, D], BF16, tag="xbig", name="xbig")
    src_r = src.rearrange("(t p) d -> p t d", p=P)
    hh = NT // 2
    nc.gpsimd.dma_start(out=xbig[:, 0:hh, :], in_=src_r[:, 0:hh, :])
    nc.gpsimd.dma_start(out=xbig[:, hh:NT, :], in_=src_r[:, hh:NT, :])
    for itg in range(NT // 4):
        xss = []
        for kk in range(4):
            it = itg * 4 + kk
            x = xbig[:, it, :]
            st = lns.tile([P, 6], F32, tag="st", name="st")
            nc.vector.bn_stats(out=st, in_=x)
            mv = lns.tile([P, 2], F32, tag="mv", name="mv")
            nc.vector.bn_aggr(out=mv, in_=st)
            inv = lns.tile([P, 1], F32, tag="inv", name="inv")
            nc.scalar.activation(
                out=inv, in_=mv[:, 1:2], func=ACTF.Sqrt, bias=eps_sb
            )
            nc.vector.reciprocal(out=inv, in_=inv)
            xs = lnxs.tile([P, D], BF16, name="xs")
            nc.vector.tensor_scalar(
                out=xs, in0=x,
                scalar1=mv[:, 0:1], scalar2=inv,
                op0=ALU.subtract, op1=ALU.mult,
            )
            xss.append(xs)
        for c in range(DC):
            ps = ps_pool.tile([P, 4 * P], BF16, tag="A", bufs=2, name="tp")
            for kk in range(4):
                nc.tensor.transpose(
                    ps[:, kk * P:(kk + 1) * P],
                    xss[kk][:, c * P:(c + 1) * P],
                    ident,
                )
            nc.scalar.copy(
                out=xT[:, c, itg * 512:(itg + 1) * 512], in_=ps)


def _body(tc, xq, xkv, wq, wkv, wout16, outs):
    nc = tc.nc

    with (
        tc.tile_pool(name="persist", bufs=1) as pers,
        tc.tile_pool(name="lns", bufs=8) as lns,
        tc.tile_pool(name="lnxs", bufs=6) as lnxs,
        tc.tile_pool(name="lnx", bufs=2) as lnx,
        tc.tile_pool(name="expp", bufs=3) as expp,
        tc.tile_pool(name="normp", bufs=1) as normp,
        tc.tile_pool(name="bigp", bufs=2) as bigp,
        tc.tile_pool(name="ps_pool", bufs=1, space="PSUM") as ps_pool,
    ):
        # ---- persistent tiles ----
        ident = pers.tile([P, P], BF16, name="ident")
        make_identity(nc, ident)
        eps_sb = pers.tile([P, 1], F32, name="eps_sb")
        nc.vector.memset(eps_sb, LN_EPS)
        ln16_sb = pers.tile([P, 1], F32, name="ln16_sb")
        nc.vector.memset(ln16_sb, float(np.log(REC_SCALE)))

        wq_sb = pers.tile([P, DC, D], BF16, name="wq_sb")
        nc.gpsimd.dma_start(out=wq_sb, in_=wq.rearrange("(c p) f -> p c f", p=P))
        wkv_sb = pers.tile([P, DC, 2 * D], BF16, name="wkv_sb")
        nc.gpsimd.dma_start(out=wkv_sb, in_=wkv.rearrange("(c p) f -> p c f", p=P))
        wout16_sb = pers.tile([P, H // 2, D], FP16, name="wout16_sb")
        nc.gpsimd.dma_start(out=wout16_sb, in_=wout16[:, :, :])

        xqT = bigp.tile([P, DC, N], BF16, tag="big", name="xqT")
        xkvT = bigp.tile([P, DC, N], BF16, tag="big", name="xkvT")
        qT = pers.tile([P, DC, N], BF16, name="qT")   # head 2c rows 0-63,
        kT = pers.tile([P, DC, N], BF16, name="kT")   # head 2c+1 rows 64-127
        # vaug fp16 [p, jt, head, col]: even heads v@0-63/ones@64-127,
        # odd heads ones@0-63/v@64-127
        vaug = pers.tile([P, NT, H, P], FP16, name="vaug")
        nc.vector.memset(vaug[:, :, 0::2, HD:P], 1.0)
        nc.vector.memset(vaug[:, :, 1::2, 0:HD], 1.0)
        # normalized attention out fp16: head 2hp rows 0-63, 2hp+1 rows 64-127
        aout16 = pers.tile([P, H // 2, N], FP16, name="aout16")

        # ---- phase A: layernorm + transpose (both streams) ----
        _phase_a(tc, lnx, lns, lnxs, ps_pool, xq, xqT, ident, eps_sb)
        _phase_a(tc, lnx, lns, lnxs, ps_pool, xkv, xkvT, ident, eps_sb)

        # ---- phase B: projections (bf16) ----
        for dst, w_sb, xT in ((qT, wq_sb, xqT), (kT, wkv_sb, xkvT)):
            for m in range(DC):
                for nch in range(4):
                    ps = ps_pool.tile([P, IC], F32, tag="A", bufs=2,
                                      name="ps")
                    for k in range(DC):
                        nc.tensor.matmul(
                            ps,
                            lhsT=w_sb[:, k, m * P:(m + 1) * P],
                            rhs=xT[:, k, nch * IC:(nch + 1) * IC],
                            start=(k == 0), stop=(k == DC - 1),
                        )
                    nc.vector.tensor_copy(
                        out=dst[:, m, nch * IC:(nch + 1) * IC], in_=ps
                    )
        # v token-major -> vaug fp16 (parity-split destinations)
        for mt in range(NT):
            ps = ps_pool.tile([P, D], F32, tag="A", bufs=2, name="psv")
            for k in range(DC):
                nc.tensor.matmul(
                    ps,
                    lhsT=xkvT[:, k, mt * P:(mt + 1) * P],
                    rhs=wkv_sb[:, k, D:2 * D],
                    start=(k == 0), stop=(k == DC - 1),
                )
            psr = ps.rearrange("p (h d) -> p h d", h=H)
            nc.scalar.copy(out=vaug[:, mt, 0::2, 0:HD], in_=psr[:, 0::2, :])
            nc.scalar.copy(out=vaug[:, mt, 1::2, HD:P], in_=psr[:, 1::2, :])

        # ---- phase C: attention ----
        # unit = (head pair hp, i-chunk iq). PSUM: simA/simB [128, 2, 512]
        # (2 banks each, jt-pair slots) + avA/avB [128, 512] (bufs=2 ring).
        # exp mix per unit: S10/D6 (even units) S9/D7 (odd).
        EXP_EVEN = [("s", "d"), ("d", "s")] * 4
        EXP_ODD = [("d", "s"), ("s", "d")] * 4

        def emit_exp(eng, ex, sim):
            sim_flat = sim.rearrange("p a b -> p (a b)")
            if eng == "s":
                nc.scalar.activation(out=ex.rearrange("p a b -> p (a b)"),
                                     in_=sim_flat, func=ACTF.Exp)
            else:
                nc.vector.tensor_scalar(
                    out=ex.rearrange("p a b -> p (a b)").bitcast(I16),
                    in0=sim_flat, scalar1=float(S16_A), scalar2=float(S16_B),
                    op0=ALU.mult, op1=ALU.add)

        for hp in range(H // 2):
            for iq in range(4):
                u = hp * 4 + iq
                sims = [
                    ps_pool.tile([P, 2, IC], F32, tag="BC"[s], bufs=1,
                                 name=f"sim{s}")
                    for s in range(2)
                ]
                avs = [
                    ps_pool.tile([P, IC], F32, tag=("A" if s == 0 else "av1"),
                                 bufs=2, name=f"av{s}")
                    for s in range(2)
                ]
                junk = expp.tile([1, 2], F32, tag="junk", bufs=2,
                                 name="junk")
                for jp in range(NT // 2):
                    exs = [
                        expp.tile([P, 2, IC], FP16, tag=f"ex{s}", bufs=2,
                                  name=f"ex{s}")
                        for s in range(2)
                    ]
                    for t in range(2):
                        jt = jp * 2 + t
                        for s in range(2):
                            hb = s * HD
                            nc.tensor.matmul(
                                sims[s][:, t, :],
                                lhsT=kT[hb:hb + HD, hp, jt * P:(jt + 1) * P],
                                rhs=qT[hb:hb + HD, hp, iq * IC:(iq + 1) * IC],
                                start=True, stop=True,
                            )
                    ea, eb = (EXP_EVEN if u % 2 == 0 else EXP_ODD)[jp]
                    emit_exp(ea, exs[0], sims[0])
                    emit_exp(eb, exs[1], sims[1])
                    # cross-dep: the next jp's FIRST sim (side 0) must wait
                    # for BOTH exps, so the row-group pair issues together
                    # (PE queue is in-order, side 1 follows immediately).
                    nc.vector.tensor_tensor(
                        out=junk, in0=sims[0][0:1, 0, 0:2],
                        in1=exs[1][0:1, 0, 0:2], op=ALU.add)
                    for t in range(2):
                        jt = jp * 2 + t
                        for s in range(2):
                            nc.tensor.matmul(
                                avs[s],
                                lhsT=vaug[:, jt, 2 * hp + s, :],
                                rhs=exs[s][:, t, :],
                                start=(jt == 0), stop=(jt == NT - 1),
                            )
                # normalization. side 0: numerator rows 0-63, D rows 64-127;
                # side 1 flipped. Ln/Exp on the D rows in place, DMA the
                # reciprocal to the numerator's partition half, multiply.
                for s in range(2):
                    dlo = (1 - s) * HD    # D rows base
                    nlo = s * HD          # numerator rows base
                    lnD = normp.tile([P, IC], F32, tag=f"lnD{s}",
                                     name="lnD")
                    nc.scalar.activation(
                        out=lnD[dlo:dlo + HD, :], in_=avs[s][dlo:dlo + HD, :],
                        func=ACTF.Ln)
                    recE = normp.tile([P, IC], F32, tag=f"recE{s}",
                                      name="recE")
                    nc.scalar.activation(
                        out=recE[dlo:dlo + HD, :], in_=lnD[dlo:dlo + HD, :],
                        func=ACTF.Exp, scale=-1.0,
                        bias=ln16_sb[dlo:dlo + HD, :])
                    recN = normp.tile([P, IC], F32, tag=f"recN{s}",
                                      name="recN")
                    nc.sync.dma_start(out=recN[nlo:nlo + HD, :],
                                      in_=recE[dlo:dlo + HD, :])
                    nc.vector.tensor_tensor(
                        out=aout16[nlo:nlo + HD, hp, iq * IC:(iq + 1) * IC],
                        in0=avs[s][nlo:nlo + HD, :],
                        in1=recN[nlo:nlo + HD, :], op=ALU.mult)

        # ---- phase D: out projection, fp16 K=128 head pairs ----
        for g in range(NT // 4):
            os_big = bigp.tile([P, 4, D], F32, tag="big", name="os_big")
            for j in range(4):
                it = g * 4 + j
                ps = ps_pool.tile([P, D], F32, tag="B", bufs=1, name="pso")
                for hp in range(H // 2):
                    nc.tensor.matmul(
                        ps,
                        lhsT=aout16[:, hp, it * P:(it + 1) * P],
                        rhs=wout16_sb[:, hp, :],
                        start=(hp == 0), stop=(hp == H // 2 - 1),
                    )
                nc.vector.tensor_scalar(
                    out=os_big[:, j, :], in0=ps,
                    scalar1=float(OUT_DESCALE), scalar2=0.0,
                    op0=ALU.mult, op1=ALU.add)
            nc.sync.dma_start(out=outs[g][:, :, :], in_=os_big)


def _get_nc():
    global _NC_CACHE
    if _NC_CACHE is None:
        _NC_CACHE = _build_program()
    return _NC_CACHE


def kernel(i, t, g_i, b_i, g_t, b_t, w_qkv_i, w_qkv_t, w_out_i, w_out_t):
    global LAST_EXEC_NS
    nc = _get_nc()

    i = np.asarray(i, np.float32)
    t = np.asarray(t, np.float32)
    bf = ml_dtypes.bfloat16
    f16 = np.float16
    w_qkv_i = np.asarray(w_qkv_i, np.float32)
    w_qkv_t = np.asarray(w_qkv_t, np.float32)
    g_i = np.asarray(g_i, np.float32)
    g_t = np.asarray(g_t, np.float32)
    assert np.abs(np.asarray(b_i)).max() == 0.0, "beta_i must be zero"
    assert np.abs(np.asarray(b_t)).max() == 0.0, "beta_t must be zero"

    # gamma folded into projection weights (exact); 0.125 folded into wq
    wq_i = (w_qkv_i[:, :D] * 0.125 * g_i[:, None]).astype(bf)
    wq_t = (w_qkv_t[:, :D] * 0.125 * g_t[:, None]).astype(bf)
    wkv_i = np.ascontiguousarray(w_qkv_i[:, D:] * g_i[:, None]).astype(bf)
    wkv_t = np.ascontiguousarray(w_qkv_t[:, D:] * g_t[:, None]).astype(bf)

    def mk_wout16(w):
        # [512, 512] -> [128, 4, 512]: rows 0-63 = head 2hp, 64-127 = 2hp+1
        w = np.asarray(w, np.float32)
        w = w.reshape(H // 2, P, D).transpose(1, 0, 2)
        return np.ascontiguousarray(w).astype(f16)

    wo16_i = mk_wout16(w_out_i)
    wo16_t = mk_wout16(w_out_t)
    f32 = lambda a: np.ascontiguousarray(np.asarray(a, np.float32))

    in_maps = []
    for c in range(8):
        b, d = c // 2, c % 2
        if d == 0:  # t -> i: queries from t, keys/values from i
            m = dict(xq=f32(t[b]), xkv=f32(i[b]),
                     wq=wq_t, wkv=wkv_i, wout16=wo16_i)
        else:       # i -> t
            m = dict(xq=f32(i[b]), xkv=f32(t[b]),
                     wq=wq_i, wkv=wkv_t, wout16=wo16_t)
        in_maps.append(m)

    res = run_bass_kernel_spmd(nc, in_maps, list(range(8)))
    LAST_EXEC_NS = res.exec_time_ns

    out = np.empty((4, 2 * N, D), np.float32)
    for c in range(8):
        b, d = c // 2, c % 2
        for g in range(NT // 4):
            blk = res.results[c][f"out{g}"]  # [128, 4, 512]
            for j in range(4):
                it = g * 4 + j
                out[b, d * N + it * P:d * N + (it + 1) * P, :] = blk[:, j, :]
    return out


# revision 17
# speedup vs baseline: 1.1793x; 1.1793x over previous
"""Trainium2 Bass kernel: dual cross-attention block (nn_CA_36670430773307).

Full-input contract: kernel(**inputs) takes the complete unsharded tensors and
returns the complete (4, 4096, 512) output.

Sharding: 8 cores = batch(4) x direction(2). Each core computes one full
cross-attention direction (t->i or i->t) for one batch element.

v3 design (HW-measured microbenchmarks behind every choice):
  - sim (q@kT): bf16, K=64 head pairs row-grouped at partitions 0-63/64-127;
    a pair issues every ~260ns (true PE row-group concurrency).
  - exp: split Scalar (native Exp, PSUM->fp16, ~1.07ns/col) and DVE
    (Schraudolph-to-fp16 bits: int16 = round(1024/ln2*x + 15*1024-44),
    bit-reinterpreted as fp16; round-to-nearest conversion confirmed).
    GPSIMD cannot read PSUM so it only triggers DMAs.
  - av (attn@v): fp16, K=128 per j-tile. vaug [128, 128] per (jt, head):
    even heads: cols 0-63 = v, 64-127 = ones; odd heads flipped. The av
    output then carries the softmax denominator replicated on the OPPOSITE
    64 partitions from the numerator, and numerators of a head pair land on
    complementary partition halves -> out-projection runs K=128.
  - softmax 1/D: Ln then Exp(-x+ln16) on Scalar (4e-5 rel err), computed on
    the D rows in place, one SBUF->SBUF DMA to shift to the numerator's
    partitions, one DVE multiply (PSUM x SBUF -> fp16 aout).
  - out projection: fp16, K=128 (head pair per matmul), x16 descale folded
    into the final PSUM->SBUF copy.
  - gamma/beta: gamma folded into the projection weights host-side (exact);
    beta is guaranteed zero by the problem spec (asserted).
Precision (numpy pipeline sim): rel err ~1.0e-2 vs the 2e-2 gate.
"""

import numpy as np
import ml_dtypes

import concourse.bass as bass
import concourse.mybir as mybir
import concourse.tile as tile
from concourse.bass_utils import run_bass_kernel_spmd
from concourse.masks import make_identity

N = 2048            # tokens per stream
D = 512             # model dim
H = 8               # heads
HD = 64             # head dim
P = 128             # SBUF partitions
NT = N // P         # 16 token tiles
DC = D // P         # 4 model-dim chunks
IC = 512            # i-chunk (PSUM bank free size fp32)
LN_EPS = 1e-5

F32 = mybir.dt.float32
BF16 = mybir.dt.bfloat16
FP16 = mybir.dt.float16
I16 = mybir.dt.int16
ALU = mybir.AluOpType
ACTF = mybir.ActivationFunctionType

S16_A = 1024.0 / np.log(2.0)   # schraudolph-to-fp16: bits = A*x + B
S16_B = 15.0 * 1024 - 44.0
REC_SCALE = 16.0               # folded into Exp(-lnD + ln REC_SCALE)
OUT_DESCALE = 1.0 / REC_SCALE

LAST_EXEC_NS = None
_NC_CACHE = None


def _legalize_waits(js):
    """Walrus encodes ONE sync wait per instruction; split extras onto
    EventSemaphore instructions on the same engine."""
    for f in js["functions"]:
        for b in f["blocks"]:
            out = []
            for ins in b["instructions"]:
                si = ins.get("sync_info") or {}
                ow = si.get("on_wait") or []
                if len(ow) > 1:
                    for k, w in enumerate(ow[:-1]):
                        out.append({
                            "debug": ins.get("debug"),
                            "engine": ins["engine"],
                            "ins": [], "outs": [],
                            "name": f"{ins['name']}_w{k}",
                            "opcode": "EventSemaphore",
                            "sync_info": {"on_update": [], "on_wait": [w]},
                        })
                    si = dict(si)
                    si["on_wait"] = [ow[-1]]
                    ins = dict(ins)
                    ins["sync_info"] = si
                out.append(ins)
            b["instructions"] = out


def _build_program():
    nc = bass.Bass()

    xq = nc.declare_dram_parameter("xq", [N, D], F32, isOutput=False)
    xkv = nc.declare_dram_parameter("xkv", [N, D], F32, isOutput=False)
    wq = nc.declare_dram_parameter("wq", [D, D], BF16, isOutput=False)
    wkv = nc.declare_dram_parameter("wkv", [D, 2 * D], BF16, isOutput=False)
    wout16 = nc.declare_dram_parameter("wout16", [P, H // 2, D], FP16,
                                       isOutput=False)
    outs = [
        nc.declare_dram_parameter(f"out{g}", [P, 4, D], F32, isOutput=True)
        for g in range(NT // 4)
    ]

    with tile.TileContext(nc) as tc:
        _body(tc, xq, xkv, wq, wkv, wout16, outs)

    import json
    js = json.loads(nc.to_json_bytes())
    _legalize_waits(js)
    legalized = json.dumps(js).encode()
    nc.to_json_bytes = lambda: legalized
    return nc


def _phase_a(tc, lnx, lns, lnxs, ps_pool, src, xT, ident, eps_sb):
    """LayerNorm one stream token-major (gamma/beta folded into weights
    host-side), PE-transpose into d-major xT."""
    nc = tc.nc
    xbig = lnx.tile([P, NT, D], BF16, tag="xbig", name="xbig")
    src_r = src.rearrange("(t p) d -> p t d", p=P)
    hh = NT // 2
    nc.gpsimd.dma_start(out=xbig[:, 0:hh, :], in_=src_r[:, 0:hh, :])
    nc.gpsimd.dma_start(out=xbig[:, hh:NT, :], in_=src_r[:, hh:NT, :])
    for itg in range(NT // 4):
        xss = []
        for kk in range(4):
            it = itg * 4 + kk
            x = xbig[:, it, :]
            st = lns.tile([P, 6], F32, tag="st", name="st")
            nc.vector.bn_stats(out=st, in_=x)
            mv = lns.tile([P, 2], F32, tag="mv", name="mv")
            nc.vector.bn_aggr(out=mv, in_=st)
            inv = lns.tile([P, 1], F32, tag="inv", name="inv")
            nc.scalar.activation(
                out=inv, in_=mv[:, 1:2], func=ACTF.Sqrt, bias=eps_sb
            )
            nc.vector.reciprocal(out=inv, in_=inv)
            xs = lnxs.tile([P, D], BF16, name="xs")
            nc.vector.tensor_scalar(
                out=xs, in0=x,
                scalar1=mv[:, 0:1], scalar2=inv,
                op0=ALU.subtract, op1=ALU.mult,
            )
            xss.append(xs)
        for c in range(DC):
            ps = ps_pool.tile([P, 4 * P], BF16, tag="A", bufs=2, name="tp")
            for kk in range(4):
                nc.tensor.transpose(
                    ps[:, kk * P:(kk + 1) * P],
                    xss[kk][:, c * P:(c + 1) * P],
                    ident,
                )
            nc.scalar.copy(
                out=xT[:, c, itg * 512:(itg + 1) * 512], in_=ps)


def _body(tc, xq, xkv, wq, wkv, wout16, outs):
    nc = tc.nc

    with (
        tc.tile_pool(name="persist", bufs=1) as pers,
        tc.tile_pool(name="lns", bufs=8) as lns,
        tc.tile_pool(name="lnxs", bufs=6) as lnxs,
        tc.tile_pool(name="lnx", bufs=2) as lnx,
        tc.tile_pool(name="expp", bufs=3) as expp,
        tc.tile_pool(name="normp", bufs=1) as normp,
        tc.tile_pool(name="bigp", bufs=2) as bigp,
        tc.tile_pool(name="ps_pool", bufs=1, space="PSUM") as ps_pool,
    ):
        # ---- persistent tiles ----
        ident = pers.tile([P, P], BF16, name="ident")
        make_identity(nc, ident)
        eps_sb = pers.tile([P, 1], F32, name="eps_sb")
        nc.vector.memset(eps_sb, LN_EPS)
        ln16_sb = pers.tile([P, 1], F32, name="ln16_sb")
        nc.vector.memset(ln16_sb, float(np.log(REC_SCALE)))

        wq_sb = pers.tile([P, DC, D], BF16, name="wq_sb")
        nc.gpsimd.dma_start(out=wq_sb, in_=wq.rearrange("(c p) f -> p c f", p=P))
        wkv_sb = pers.tile([P, DC, 2 * D], BF16, name="wkv_sb")
        nc.gpsimd.dma_start(out=wkv_sb, in_=wkv.rearrange("(c p) f -> p c f", p=P))
        wout16_sb = pers.tile([P, H // 2, D], FP16, name="wout16_sb")
        nc.gpsimd.dma_start(out=wout16_sb, in_=wout16[:, :, :])

        xqT = bigp.tile([P, DC, N], BF16, tag="big", name="xqT")
        xkvT = bigp.tile([P, DC, N], BF16, tag="big", name="xkvT")
        qT = pers.tile([P, DC, N], BF16, name="qT")   # head 2c rows 0-63,
        kT = pers.tile([P, DC, N], BF16, name="kT")   # head 2c+1 rows 64-127
        # vaug fp16 [p, jt, head, col]: even heads v@0-63/ones@64-127,
        # odd heads ones@0-63/v@64-127
        vaug = pers.tile([P, NT, H, P], FP16, name="vaug")
        nc.vector.memset(vaug[:, :, 0::2, HD:P], 1.0)
        nc.vector.memset(vaug[:, :, 1::2, 0:HD], 1.0)
        # normalized attention out fp16: head 2hp rows 0-63, 2hp+1 rows 64-127
        aout16 = pers.tile([P, H // 2, N], FP16, name="aout16")

        # ---- phase A: layernorm + transpose (both streams) ----
        _phase_a(tc, lnx, lns, lnxs, ps_pool, xq, xqT, ident, eps_sb)
        _phase_a(tc, lnx, lns, lnxs, ps_pool, xkv, xkvT, ident, eps_sb)

        # ---- phase B: projections (bf16) ----
        for dst, w_sb, xT in ((qT, wq_sb, xqT), (kT, wkv_sb, xkvT)):
            for m in range(DC):
                for nch in range(4):
                    ps = ps_pool.tile([P, IC], F32, tag="A", bufs=2,
                                      name="ps")
                    for k in range(DC):
                        nc.tensor.matmul(
                            ps,
                            lhsT=w_sb[:, k, m * P:(m + 1) * P],
                            rhs=xT[:, k, nch * IC:(nch + 1) * IC],
                            start=(k == 0), stop=(k == DC - 1),
                        )
                    nc.vector.tensor_copy(
                        out=dst[:, m, nch * IC:(nch + 1) * IC], in_=ps
                    )
        # v token-major -> vaug fp16 (parity-split destinations)
        for mt in range(NT):
            ps = ps_pool.tile([P, D], F32, tag="A", bufs=2, name="psv")
            for k in range(DC):
                nc.tensor.matmul(
                    ps,
                    lhsT=xkvT[:, k, mt * P:(mt + 1) * P],
                    rhs=wkv_sb[:, k, D:2 * D],
                    start=(k == 0), stop=(k == DC - 1),
                )
            psr = ps.rearrange("p (h d) -> p h d", h=H)
            nc.scalar.copy(out=vaug[:, mt, 0::2, 0:HD], in_=psr[:, 0::2, :])
            nc.scalar.copy(out=vaug[:, mt, 1::2, HD:P], in_=psr[:, 1::2, :])

        # ---- phase C: attention ----
        # unit = (head pair hp, i-chunk iq). PSUM: simA/simB [128, 2, 512]
        # (2 banks each, jt-pair slots) + avA/avB [128, 512] (bufs=2 ring).
        # exp mix per unit: S10/D6 (even units) S9/D7 (odd).
        EXP_EVEN = [("s", "d"), ("d", "s")] * 4
        EXP_ODD = [("d", "s"), ("s", "d")] * 4

        def emit_exp(eng, ex, sim):
            sim_flat = sim.rearrange("p a b -> p (a b)")
            if eng == "s":
                nc.scalar.activation(out=ex.rearrange("p a b -> p (a b)"),
                                     in_=sim_flat, func=ACTF.Exp)
            else:
                nc.vector.tensor_scalar(
                    out=ex.rearrange("p a b -> p (a b)").bitcast(I16),
                    in0=sim_flat, scalar1=float(S16_A), scalar2=float(S16_B),
                    op0=ALU.mult, op1=ALU.add)

        for hp in range(H // 2):
            for iq in range(4):
                u = hp * 4 + iq
                avs = [
                    ps_pool.tile([P, IC], F32, tag=("A" if s == 0 else "av1"),
                                 bufs=2, name=f"av{s}")
                    for s in range(2)
                ]
                for jt in range(NT):
                    sim = ps_pool.tile([P, 2, IC], F32, tag="SIM", bufs=2,
                                       name="sim")
                    for s in range(2):
                        hb = s * HD
                        nc.tensor.matmul(
                            sim[:, s, :],
                            lhsT=kT[hb:hb + HD, hp, jt * P:(jt + 1) * P],
                            rhs=qT[hb:hb + HD, hp, iq * IC:(iq + 1) * IC],
                            start=True, stop=True,
                        )
                    ex = expp.tile([P, 2, IC], FP16, tag="ex", bufs=4,
                                   name="ex")
                    emit_exp(("s", "d")[(jt + u) % 2], ex, sim)
                    for s in range(2):
                        nc.tensor.matmul(
                            avs[s],
                            lhsT=vaug[:, jt, 2 * hp + s, :],
                            rhs=ex[:, s, :],
                            start=(jt == 0), stop=(jt == NT - 1),
                        )
                # normalization. side 0: numerator rows 0-63, D rows 64-127;
                # side 1 flipped. Ln/Exp on the D rows in place, DMA the
                # reciprocal to the numerator's partition half, multiply.
                for s in range(2):
                    dlo = (1 - s) * HD    # D rows base
                    nlo = s * HD          # numerator rows base
                    lnD = normp.tile([P, IC], F32, tag=f"lnD{s}",
                                     name="lnD")
                    nc.scalar.activation(
                        out=lnD[dlo:dlo + HD, :], in_=avs[s][dlo:dlo + HD, :],
                        func=ACTF.Ln)
                    recE = normp.tile([P, IC], F32, tag=f"recE{s}",
                                      name="recE")
                    nc.scalar.activation(
                        out=recE[dlo:dlo + HD, :], in_=lnD[dlo:dlo + HD, :],
                        func=ACTF.Exp, scale=-1.0,
                        bias=ln16_sb[dlo:dlo + HD, :])
                    recN = normp.tile([P, IC], F32, tag=f"recN{s}",
                                      name="recN")
                    nc.sync.dma_start(out=recN[nlo:nlo + HD, :],
                                      in_=recE[dlo:dlo + HD, :])
                    nc.vector.tensor_tensor(
                        out=aout16[nlo:nlo + HD, hp, iq * IC:(iq + 1) * IC],
                        in0=avs[s][nlo:nlo + HD, :],
                        in1=recN[nlo:nlo + HD, :], op=ALU.mult)

        # ---- phase D: out projection, fp16 K=128 head pairs ----
        for g in range(NT // 4):
            os_big = bigp.tile([P, 4, D], F32, tag="big", name="os_big")
            for j in range(4):
                it = g * 4 + j
                ps = ps_pool.tile([P, D], F32, tag="av1", bufs=2, name="pso")
                for hp in range(H // 2):
                    nc.tensor.matmul(
                        ps,
                        lhsT=aout16[:, hp, it * P:(it + 1) * P],
                        rhs=wout16_sb[:, hp, :],
                        start=(hp == 0), stop=(hp == H // 2 - 1),
                    )
                nc.vector.tensor_scalar(
                    out=os_big[:, j, :], in0=ps,
                    scalar1=float(OUT_DESCALE), scalar2=0.0,
                    op0=ALU.mult, op1=ALU.add)
            nc.sync.dma_start(out=outs[g][:, :, :], in_=os_big)


def _get_nc():
    global _NC_CACHE
    if _NC_CACHE is None:
        _NC_CACHE = _build_program()
    return _NC_CACHE


def kernel(i, t, g_i, b_i, g_t, b_t, w_qkv_i, w_qkv_t, w_out_i, w_out_t):
    global LAST_EXEC_NS
    nc = _get_nc()

    i = np.asarray(i, np.float32)
    t = np.asarray(t, np.float32)
    bf = ml_dtypes.bfloat16
    f16 = np.float16
    w_qkv_i = np.asarray(w_qkv_i, np.float32)
    w_qkv_t = np.asarray(w_qkv_t, np.float32)
    g_i = np.asarray(g_i, np.float32)
    g_t = np.asarray(g_t, np.float32)
    assert np.abs(np.asarray(b_i)).max() == 0.0, "beta_i must be zero"
    assert np.abs(np.asarray(b_t)).max() == 0.0, "beta_t must be zero"

    # gamma folded into projection weights (exact); 0.125 folded into wq
    wq_i = (w_qkv_i[:, :D] * 0.125 * g_i[:, None]).astype(bf)
    wq_t = (w_qkv_t[:, :D] * 0.125 * g_t[:, None]).astype(bf)
    wkv_i = np.ascontiguousarray(w_qkv_i[:, D:] * g_i[:, None]).astype(bf)
    wkv_t = np.ascontiguousarray(w_qkv_t[:, D:] * g_t[:, None]).astype(bf)

    def mk_wout16(w):
        # [512, 512] -> [128, 4, 512]: rows 0-63 = head 2hp, 64-127 = 2hp+1
        w = np.asarray(w, np.float32)
        w = w.reshape(H // 2, P, D).transpose(1, 0, 2)
        return np.ascontiguousarray(w).astype(f16)

    wo16_i = mk_wout16(w_out_i)
    wo16_t = mk_wout16(w_out_t)
    f32 = lambda a: np.ascontiguousarray(np.asarray(a, np.float32))

    in_maps = []
    for c in range(8):
        b, d = c // 2, c % 2
        if d == 0:  # t -> i: queries from t, keys/values from i
            m = dict(xq=f32(t[b]), xkv=f32(i[b]),
                     wq=wq_t, wkv=wkv_i, wout16=wo16_i)
        else:       # i -> t
            m = dict(xq=f32(i[b]), xkv=f32(t[b]),
                     wq=wq_i, wkv=wkv_t, wout16=wo16_t)
        in_maps.append(m)

    res = run_bass_kernel_spmd(nc, in_maps, list(range(8)))
    LAST_EXEC_NS = res.exec_time_ns

    out = np.empty((4, 2 * N, D), np.float32)
    for c in range(8):
        b, d = c // 2, c % 2
        for g in range(NT // 4):
            blk = res.results[c][f"out{g}"]  # [128, 4, 512]
            for j in range(4):
                it = g * 4 + j
                out[b, d * N + it * P:d * N + (it + 1) * P, :] = blk[:, j, :]
    return out


# revision 18
# speedup vs baseline: 1.2230x; 1.0371x over previous
"""Trainium2 Bass kernel: dual cross-attention block (nn_CA_36670430773307).

Full-input contract: kernel(**inputs) takes the complete unsharded tensors and
returns the complete (4, 4096, 512) output.

Sharding: 8 cores = batch(4) x direction(2). Each core computes one full
cross-attention direction (t->i or i->t) for one batch element.

v3 design (HW-measured microbenchmarks behind every choice):
  - sim (q@kT): bf16, K=64 head pairs row-grouped at partitions 0-63/64-127;
    a pair issues every ~260ns (true PE row-group concurrency).
  - exp: split Scalar (native Exp, PSUM->fp16, ~1.07ns/col) and DVE
    (Schraudolph-to-fp16 bits: int16 = round(1024/ln2*x + 15*1024-44),
    bit-reinterpreted as fp16; round-to-nearest conversion confirmed).
    GPSIMD cannot read PSUM so it only triggers DMAs.
  - av (attn@v): fp16, K=128 per j-tile. vaug [128, 128] per (jt, head):
    even heads: cols 0-63 = v, 64-127 = ones; odd heads flipped. The av
    output then carries the softmax denominator replicated on the OPPOSITE
    64 partitions from the numerator, and numerators of a head pair land on
    complementary partition halves -> out-projection runs K=128.
  - softmax 1/D: Ln then Exp(-x+ln16) on Scalar (4e-5 rel err), computed on
    the D rows in place, one SBUF->SBUF DMA to shift to the numerator's
    partitions, one DVE multiply (PSUM x SBUF -> fp16 aout).
  - out projection: fp16, K=128 (head pair per matmul), x16 descale folded
    into the final PSUM->SBUF copy.
  - gamma/beta: gamma folded into the projection weights host-side (exact);
    beta is guaranteed zero by the problem spec (asserted).
Precision (numpy pipeline sim): rel err ~1.0e-2 vs the 2e-2 gate.
"""

import numpy as np
import ml_dtypes

import concourse.bass as bass
import concourse.mybir as mybir
import concourse.tile as tile
from concourse.bass_utils import run_bass_kernel_spmd
from concourse.masks import make_identity

N = 2048            # tokens per stream
D = 512             # model dim
H = 8               # heads
HD = 64             # head dim
P = 128             # SBUF partitions
NT = N // P         # 16 token tiles
DC = D // P         # 4 model-dim chunks
IC = 512            # i-chunk (PSUM bank free size fp32)
LN_EPS = 1e-5

F32 = mybir.dt.float32
BF16 = mybir.dt.bfloat16
FP16 = mybir.dt.float16
I16 = mybir.dt.int16
ALU = mybir.AluOpType
ACTF = mybir.ActivationFunctionType

S16_A = 1024.0 / np.log(2.0)   # schraudolph-to-fp16: bits = A*x + B
S16_B = 15.0 * 1024 - 44.0
REC_SCALE = 16.0               # folded into Exp(-lnD + ln REC_SCALE)
OUT_DESCALE = 1.0 / REC_SCALE

LAST_EXEC_NS = None
_NC_CACHE = None


def _legalize_waits(js):
    """Walrus encodes ONE sync wait per instruction; split extras onto
    EventSemaphore instructions on the same engine."""
    for f in js["functions"]:
        for b in f["blocks"]:
            out = []
            for ins in b["instructions"]:
                si = ins.get("sync_info") or {}
                ow = si.get("on_wait") or []
                if len(ow) > 1:
                    for k, w in enumerate(ow[:-1]):
                        out.append({
                            "debug": ins.get("debug"),
                            "engine": ins["engine"],
                            "ins": [], "outs": [],
                            "name": f"{ins['name']}_w{k}",
                            "opcode": "EventSemaphore",
                            "sync_info": {"on_update": [], "on_wait": [w]},
                        })
                    si = dict(si)
                    si["on_wait"] = [ow[-1]]
                    ins = dict(ins)
                    ins["sync_info"] = si
                out.append(ins)
            b["instructions"] = out


def _build_program():
    nc = bass.Bass()

    xq = nc.declare_dram_parameter("xq", [N, D], F32, isOutput=False)
    xkv = nc.declare_dram_parameter("xkv", [N, D], F32, isOutput=False)
    wq = nc.declare_dram_parameter("wq", [D, D], BF16, isOutput=False)
    wkv = nc.declare_dram_parameter("wkv", [D, 2 * D], BF16, isOutput=False)
    wout16 = nc.declare_dram_parameter("wout16", [P, H // 2, D], FP16,
                                       isOutput=False)
    outs = [
        nc.declare_dram_parameter(f"out{g}", [P, 4, D], F32, isOutput=True)
        for g in range(NT // 4)
    ]

    with tile.TileContext(nc) as tc:
        _body(tc, xq, xkv, wq, wkv, wout16, outs)

    import json
    js = json.loads(nc.to_json_bytes())
    _legalize_waits(js)
    legalized = json.dumps(js).encode()
    nc.to_json_bytes = lambda: legalized
    return nc


def _phase_a(tc, lnx, lns, lnxs, ps_pool, src, xT, ident, eps_sb):
    """LayerNorm one stream token-major (gamma/beta folded into weights
    host-side), PE-transpose into d-major xT."""
    nc = tc.nc
    xbig = lnx.tile([P, NT, D], BF16, tag="xbig", name="xbig")
    src_r = src.rearrange("(t p) d -> p t d", p=P)
    hh = NT // 2
    nc.gpsimd.dma_start(out=xbig[:, 0:hh, :], in_=src_r[:, 0:hh, :])
    nc.gpsimd.dma_start(out=xbig[:, hh:NT, :], in_=src_r[:, hh:NT, :])
    for itg in range(NT // 4):
        xss = []
        for kk in range(4):
            it = itg * 4 + kk
            x = xbig[:, it, :]
            st = lns.tile([P, 6], F32, tag="st", name="st")
            nc.vector.bn_stats(out=st, in_=x)
            mv = lns.tile([P, 2], F32, tag="mv", name="mv")
            nc.vector.bn_aggr(out=mv, in_=st)
            inv = lns.tile([P, 1], F32, tag="inv", name="inv")
            nc.scalar.activation(
                out=inv, in_=mv[:, 1:2], func=ACTF.Sqrt, bias=eps_sb
            )
            nc.vector.reciprocal(out=inv, in_=inv)
            xs = lnxs.tile([P, D], BF16, name="xs")
            nc.vector.tensor_scalar(
                out=xs, in0=x,
                scalar1=mv[:, 0:1], scalar2=inv,
                op0=ALU.subtract, op1=ALU.mult,
            )
            xss.append(xs)
        for c in range(DC):
            ps = ps_pool.tile([P, 4 * P], BF16, tag="A", bufs=2, name="tp")
            for kk in range(4):
                nc.tensor.transpose(
                    ps[:, kk * P:(kk + 1) * P],
                    xss[kk][:, c * P:(c + 1) * P],
                    ident,
                )
            nc.scalar.copy(
                out=xT[:, c, itg * 512:(itg + 1) * 512], in_=ps)


def _body(tc, xq, xkv, wq, wkv, wout16, outs):
    nc = tc.nc

    with (
        tc.tile_pool(name="persist", bufs=1) as pers,
        tc.tile_pool(name="lns", bufs=8) as lns,
        tc.tile_pool(name="lnxs", bufs=6) as lnxs,
        tc.tile_pool(name="lnx", bufs=2) as lnx,
        tc.tile_pool(name="expp", bufs=3) as expp,
        tc.tile_pool(name="normp", bufs=1) as normp,
        tc.tile_pool(name="bigp", bufs=2) as bigp,
        tc.tile_pool(name="ps_pool", bufs=1, space="PSUM") as ps_pool,
    ):
        # ---- persistent tiles ----
        ident = pers.tile([P, P], BF16, name="ident")
        make_identity(nc, ident)
        eps_sb = pers.tile([P, 1], F32, name="eps_sb")
        nc.vector.memset(eps_sb, LN_EPS)
        ln16_sb = pers.tile([P, 1], F32, name="ln16_sb")
        nc.vector.memset(ln16_sb, float(np.log(REC_SCALE)))

        wq_sb = pers.tile([P, DC, D], BF16, name="wq_sb")
        nc.gpsimd.dma_start(out=wq_sb, in_=wq.rearrange("(c p) f -> p c f", p=P))
        wkv_sb = pers.tile([P, DC, 2 * D], BF16, name="wkv_sb")
        nc.gpsimd.dma_start(out=wkv_sb, in_=wkv.rearrange("(c p) f -> p c f", p=P))
        wout16_sb = pers.tile([P, H // 2, D], FP16, name="wout16_sb")
        nc.gpsimd.dma_start(out=wout16_sb, in_=wout16[:, :, :])

        xqT = bigp.tile([P, DC, N], BF16, tag="big", name="xqT")
        xkvT = bigp.tile([P, DC, N], BF16, tag="big", name="xkvT")
        qT = pers.tile([P, DC, N], BF16, name="qT")   # head 2c rows 0-63,
        kT = pers.tile([P, DC, N], BF16, name="kT")   # head 2c+1 rows 64-127
        # vaug fp16 [p, jt, head, col]: even heads v@0-63/ones@64-127,
        # odd heads ones@0-63/v@64-127
        vaug = pers.tile([P, NT, H, P], FP16, name="vaug")
        nc.vector.memset(vaug[:, :, 0::2, HD:P], 1.0)
        nc.vector.memset(vaug[:, :, 1::2, 0:HD], 1.0)
        # normalized attention out fp16: head 2hp rows 0-63, 2hp+1 rows 64-127
        aout16 = pers.tile([P, H // 2, N], FP16, name="aout16")

        # ---- phase A: layernorm + transpose (both streams) ----
        _phase_a(tc, lnx, lns, lnxs, ps_pool, xq, xqT, ident, eps_sb)
        _phase_a(tc, lnx, lns, lnxs, ps_pool, xkv, xkvT, ident, eps_sb)

        # ---- phase B: projections (bf16) ----
        for dst, w_sb, xT in ((qT, wq_sb, xqT), (kT, wkv_sb, xkvT)):
            for m in range(DC):
                for nch in range(4):
                    ps = ps_pool.tile([P, IC], F32, tag="A", bufs=2,
                                      name="ps")
                    for k in range(DC):
                        nc.tensor.matmul(
                            ps,
                            lhsT=w_sb[:, k, m * P:(m + 1) * P],
                            rhs=xT[:, k, nch * IC:(nch + 1) * IC],
                            start=(k == 0), stop=(k == DC - 1),
                        )
                    nc.vector.tensor_copy(
                        out=dst[:, m, nch * IC:(nch + 1) * IC], in_=ps
                    )
        # v token-major -> vaug fp16 (parity-split destinations)
        for mt in range(NT):
            ps = ps_pool.tile([P, D], F32, tag="A", bufs=2, name="psv")
            for k in range(DC):
                nc.tensor.matmul(
                    ps,
                    lhsT=xkvT[:, k, mt * P:(mt + 1) * P],
                    rhs=wkv_sb[:, k, D:2 * D],
                    start=(k == 0), stop=(k == DC - 1),
                )
            psr = ps.rearrange("p (h d) -> p h d", h=H)
            nc.scalar.copy(out=vaug[:, mt, 0::2, 0:HD], in_=psr[:, 0::2, :])
            nc.scalar.copy(out=vaug[:, mt, 1::2, HD:P], in_=psr[:, 1::2, :])

        # ---- phase C: attention ----
        # unit = (head pair hp, i-chunk iq). PSUM: simA/simB [128, 2, 512]
        # (2 banks each, jt-pair slots) + avA/avB [128, 512] (bufs=2 ring).
        # exp mix per unit: S10/D6 (even units) S9/D7 (odd).
        EXP_EVEN = [("s", "d"), ("d", "s")] * 4
        EXP_ODD = [("d", "s"), ("s", "d")] * 4

        def emit_exp(eng, ex, sim):
            sim_flat = sim.rearrange("p a b -> p (a b)")
            if eng == "s":
                nc.scalar.activation(out=ex.rearrange("p a b -> p (a b)"),
                                     in_=sim_flat, func=ACTF.Exp)
            else:
                nc.vector.tensor_scalar(
                    out=ex.rearrange("p a b -> p (a b)").bitcast(I16),
                    in0=sim_flat, scalar1=float(S16_A), scalar2=float(S16_B),
                    op0=ALU.mult, op1=ALU.add)

        for hp in range(H // 2):
            for iq in range(4):
                u = hp * 4 + iq
                avs = [
                    ps_pool.tile([P, IC], F32, tag=("A" if s == 0 else "av1"),
                                 bufs=2, name=f"av{s}")
                    for s in range(2)
                ]
                pend = []   # software pipeline: av lags sim by one jt
                for jt in range(NT + 1):
                    if jt < NT:
                        sim = ps_pool.tile([P, 2, IC], F32, tag="SIM",
                                           bufs=2, name="sim")
                        for s in range(2):
                            hb = s * HD
                            nc.tensor.matmul(
                                sim[:, s, :],
                                lhsT=kT[hb:hb + HD, hp,
                                        jt * P:(jt + 1) * P],
                                rhs=qT[hb:hb + HD, hp,
                                       iq * IC:(iq + 1) * IC],
                                start=True, stop=True,
                            )
                        ex = expp.tile([P, 2, IC], FP16, tag="ex", bufs=4,
                                       name="ex")
                        emit_exp(("s", "d")[(jt + u) % 2], ex, sim)
                        pend.append((jt, ex))
                    if jt >= 1:
                        pjt, pex = pend.pop(0)
                        for s in range(2):
                            nc.tensor.matmul(
                                avs[s],
                                lhsT=vaug[:, pjt, 2 * hp + s, :],
                                rhs=pex[:, s, :],
                                start=(pjt == 0), stop=(pjt == NT - 1),
                            )
                # normalization. side 0: numerator rows 0-63, D rows 64-127;
                # side 1 flipped. Ln/Exp on the D rows in place, DMA the
                # reciprocal to the numerator's partition half, multiply.
                for s in range(2):
                    dlo = (1 - s) * HD    # D rows base
                    nlo = s * HD          # numerator rows base
                    lnD = normp.tile([P, IC], F32, tag=f"lnD{s}",
                                     name="lnD")
                    nc.scalar.activation(
                        out=lnD[dlo:dlo + HD, :], in_=avs[s][dlo:dlo + HD, :],
                        func=ACTF.Ln)
                    recE = normp.tile([P, IC], F32, tag=f"recE{s}",
                                      name="recE")
                    nc.scalar.activation(
                        out=recE[dlo:dlo + HD, :], in_=lnD[dlo:dlo + HD, :],
                        func=ACTF.Exp, scale=-1.0,
                        bias=ln16_sb[dlo:dlo + HD, :])
                    recN = normp.tile([P, IC], F32, tag=f"recN{s}",
                                      name="recN")
                    nc.sync.dma_start(out=recN[nlo:nlo + HD, :],
                                      in_=recE[dlo:dlo + HD, :])
                    nc.vector.tensor_tensor(
                        out=aout16[nlo:nlo + HD, hp, iq * IC:(iq + 1) * IC],
                        in0=avs[s][nlo:nlo + HD, :],
                        in1=recN[nlo:nlo + HD, :], op=ALU.mult)

        # ---- phase D: out projection, fp16 K=128 head pairs ----
        for g in range(NT // 4):
            os_big = bigp.tile([P, 4, D], F32, tag="big", name="os_big")
            for j in range(4):
                it = g * 4 + j
                ps = ps_pool.tile([P, D], F32, tag="av1", bufs=2, name="pso")
                for hp in range(H // 2):
                    nc.tensor.matmul(
                        ps,
                        lhsT=aout16[:, hp, it * P:(it + 1) * P],
                        rhs=wout16_sb[:, hp, :],
                        start=(hp == 0), stop=(hp == H // 2 - 1),
                    )
                nc.vector.tensor_scalar(
                    out=os_big[:, j, :], in0=ps,
                    scalar1=float(OUT_DESCALE), scalar2=0.0,
                    op0=ALU.mult, op1=ALU.add)
            nc.sync.dma_start(out=outs[g][:, :, :], in_=os_big)


def _get_nc():
    global _NC_CACHE
    if _NC_CACHE is None:
        _NC_CACHE = _build_program()
    return _NC_CACHE


def kernel(i, t, g_i, b_i, g_t, b_t, w_qkv_i, w_qkv_t, w_out_i, w_out_t):
    global LAST_EXEC_NS
    nc = _get_nc()

    i = np.asarray(i, np.float32)
    t = np.asarray(t, np.float32)
    bf = ml_dtypes.bfloat16
    f16 = np.float16
    w_qkv_i = np.asarray(w_qkv_i, np.float32)
    w_qkv_t = np.asarray(w_qkv_t, np.float32)
    g_i = np.asarray(g_i, np.float32)
    g_t = np.asarray(g_t, np.float32)
    assert np.abs(np.asarray(b_i)).max() == 0.0, "beta_i must be zero"
    assert np.abs(np.asarray(b_t)).max() == 0.0, "beta_t must be zero"

    # gamma folded into projection weights (exact); 0.125 folded into wq
    wq_i = (w_qkv_i[:, :D] * 0.125 * g_i[:, None]).astype(bf)
    wq_t = (w_qkv_t[:, :D] * 0.125 * g_t[:, None]).astype(bf)
    wkv_i = np.ascontiguousarray(w_qkv_i[:, D:] * g_i[:, None]).astype(bf)
    wkv_t = np.ascontiguousarray(w_qkv_t[:, D:] * g_t[:, None]).astype(bf)

    def mk_wout16(w):
        # [512, 512] -> [128, 4, 512]: rows 0-63 = head 2hp, 64-127 = 2hp+1
        w = np.asarray(w, np.float32)
        w = w.reshape(H // 2, P, D).transpose(1, 0, 2)
        return np.ascontiguousarray(w).astype(f16)

    wo16_i = mk_wout16(w_out_i)
    wo16_t = mk_wout16(w_out_t)
    f32 = lambda a: np.ascontiguousarray(np.asarray(a, np.float32))

    in_maps = []
    for c in range(8):
        b, d = c // 2, c % 2
        if d == 0:  # t -> i: queries from t, keys/values from i
            m = dict(xq=f32(t[b]), xkv=f32(i[b]),
                     wq=wq_t, wkv=wkv_i, wout16=wo16_i)
        else:       # i -> t
            m = dict(xq=f32(i[b]), xkv=f32(t[b]),
                     wq=wq_i, wkv=wkv_t, wout16=wo16_t)
        in_maps.append(m)

    res = run_bass_kernel_spmd(nc, in_maps, list(range(8)))
    LAST_EXEC_NS = res.exec_time_ns

    out = np.empty((4, 2 * N, D), np.float32)
    for c in range(8):
        b, d = c // 2, c % 2
        for g in range(NT // 4):
            blk = res.results[c][f"out{g}"]  # [128, 4, 512]
            for j in range(4):
                it = g * 4 + j
                out[b, d * N + it * P:d * N + (it + 1) * P, :] = blk[:, j, :]
    return out


# revision 30
# speedup vs baseline: 1.5085x; 1.2335x over previous
"""Trainium2 Bass kernel: dual cross-attention block (nn_CA_36670430773307).

Full-input contract: kernel(**inputs) takes the complete unsharded tensors and
returns the complete (4, 4096, 512) output.

Sharding: 8 cores = batch(4) x direction(2). Each core computes one full
cross-attention direction (t->i or i->t) for one batch element.

v3 design (HW-measured microbenchmarks behind every choice):
  - sim (q@kT): bf16, K=64 head pairs row-grouped at partitions 0-63/64-127;
    a pair issues every ~260ns (true PE row-group concurrency).
  - exp: split Scalar (native Exp, PSUM->fp16, ~1.07ns/col) and DVE
    (Schraudolph-to-fp16 bits: int16 = round(1024/ln2*x + 15*1024-44),
    bit-reinterpreted as fp16; round-to-nearest conversion confirmed).
    GPSIMD cannot read PSUM so it only triggers DMAs.
  - av (attn@v): fp16, K=128 per j-tile. vaug [128, 128] per (jt, head):
    even heads: cols 0-63 = v, 64-127 = ones; odd heads flipped. The av
    output then carries the softmax denominator replicated on the OPPOSITE
    64 partitions from the numerator, and numerators of a head pair land on
    complementary partition halves -> out-projection runs K=128.
  - softmax 1/D: Ln then Exp(-x+ln16) on Scalar (4e-5 rel err), computed on
    the D rows in place, one SBUF->SBUF DMA to shift to the numerator's
    partitions, one DVE multiply (PSUM x SBUF -> fp16 aout).
  - out projection: fp16, K=128 (head pair per matmul), x16 descale folded
    into the final PSUM->SBUF copy.
  - gamma/beta: gamma folded into the projection weights host-side (exact);
    beta is guaranteed zero by the problem spec (asserted).
Precision (numpy pipeline sim): rel err ~1.0e-2 vs the 2e-2 gate.
"""

import numpy as np
import ml_dtypes

import concourse.bass as bass
import concourse.mybir as mybir
import concourse.tile as tile
from concourse.bass_utils import run_bass_kernel_spmd
from concourse.masks import make_identity

N = 2048            # tokens per stream
D = 512             # model dim
H = 8               # heads
HD = 64             # head dim
P = 128             # SBUF partitions
NT = N // P         # 16 token tiles
DC = D // P         # 4 model-dim chunks
IC = 512            # i-chunk (PSUM bank free size fp32)
LN_EPS = 1e-5

F32 = mybir.dt.float32
BF16 = mybir.dt.bfloat16
FP16 = mybir.dt.float16
I16 = mybir.dt.int16
ALU = mybir.AluOpType
ACTF = mybir.ActivationFunctionType

S16_A = 1024.0 / np.log(2.0)   # schraudolph-to-fp16: bits = A*x + B
S16_B = 15.0 * 1024 - 44.0
REC_SCALE = 16.0               # folded into Exp(-lnD + ln REC_SCALE)
OUT_DESCALE = 1.0 / REC_SCALE

LAST_EXEC_NS = None
_NC_CACHE = None


def _legalize_waits(js):
    """Walrus encodes ONE sync wait per instruction; split extras onto
    EventSemaphore instructions on the same engine."""
    for f in js["functions"]:
        for b in f["blocks"]:
            out = []
            for ins in b["instructions"]:
                si = ins.get("sync_info") or {}
                ow = si.get("on_wait") or []
                if len(ow) > 1:
                    for k, w in enumerate(ow[:-1]):
                        out.append({
                            "debug": ins.get("debug"),
                            "engine": ins["engine"],
                            "ins": [], "outs": [],
                            "name": f"{ins['name']}_w{k}",
                            "opcode": "EventSemaphore",
                            "sync_info": {"on_update": [], "on_wait": [w]},
                        })
                    si = dict(si)
                    si["on_wait"] = [ow[-1]]
                    ins = dict(ins)
                    ins["sync_info"] = si
                out.append(ins)
            b["instructions"] = out


def _build_program():
    nc = bass.Bass()

    xq = nc.declare_dram_parameter("xq", [N, D], F32, isOutput=False)
    xkv = nc.declare_dram_parameter("xkv", [N, D], F32, isOutput=False)
    wq = nc.declare_dram_parameter("wq", [D, D], BF16, isOutput=False)
    wkv = nc.declare_dram_parameter("wkv", [D, 2 * D], BF16, isOutput=False)
    wout16 = nc.declare_dram_parameter("wout16", [P, H // 2, D], FP16,
                                       isOutput=False)
    outs = [
        nc.declare_dram_parameter(f"out{g}", [P, 4, D], F32, isOutput=True)
        for g in range(NT // 4)
    ]

    with tile.TileContext(nc) as tc:
        _body(tc, xq, xkv, wq, wkv, wout16, outs)

    import json
    js = json.loads(nc.to_json_bytes())
    _legalize_waits(js)
    legalized = json.dumps(js).encode()
    nc.to_json_bytes = lambda: legalized
    return nc


def _phase_a(tc, lnx, lns, lnxs, ps_pool, src, xT, ident, eps_sb):
    """LayerNorm one stream token-major (gamma/beta folded into weights
    host-side), PE-transpose into d-major xT."""
    nc = tc.nc
    xbig = lnx.tile([P, NT, D], BF16, tag="xbig", name="xbig")
    src_r = src.rearrange("(t p) d -> p t d", p=P)
    qq = NT // 4
    for q in range(4):
        nc.gpsimd.dma_start(out=xbig[:, q * qq:(q + 1) * qq, :],
                            in_=src_r[:, q * qq:(q + 1) * qq, :])
    for itg in range(NT // 4):
        xss = []
        for kk in range(4):
            it = itg * 4 + kk
            x = xbig[:, it, :]
            st = lns.tile([P, 6], F32, tag="st", name="st")
            nc.vector.bn_stats(out=st, in_=x)
            mv = lns.tile([P, 2], F32, tag="mv", name="mv")
            nc.vector.bn_aggr(out=mv, in_=st)
            inv = lns.tile([P, 1], F32, tag="inv", name="inv")
            nc.scalar.activation(
                out=inv, in_=mv[:, 1:2], func=ACTF.Sqrt, bias=eps_sb
            )
            nc.vector.reciprocal(out=inv, in_=inv)
            xs = lnxs.tile([P, D], BF16, name="xs")
            nc.vector.tensor_scalar(
                out=xs, in0=x,
                scalar1=mv[:, 0:1], scalar2=inv,
                op0=ALU.subtract, op1=ALU.mult,
            )
            xss.append(xs)
        for c in range(DC):
            ps = ps_pool.tile([P, 4 * P], BF16, tag="A", bufs=2, name="tp")
            for kk in range(4):
                nc.tensor.transpose(
                    ps[:, kk * P:(kk + 1) * P],
                    xss[kk][:, c * P:(c + 1) * P],
                    ident,
                )
            nc.scalar.copy(
                out=xT[:, c, itg * 512:(itg + 1) * 512], in_=ps)


def _body(tc, xq, xkv, wq, wkv, wout16, outs):
    nc = tc.nc

    with (
        tc.tile_pool(name="persist", bufs=1) as pers,
        tc.tile_pool(name="lns", bufs=16) as lns,
        tc.tile_pool(name="lnxs", bufs=12) as lnxs,
        tc.tile_pool(name="lnx", bufs=2) as lnx,
        tc.tile_pool(name="expp", bufs=3) as expp,
        tc.tile_pool(name="normp", bufs=2) as normp,
        tc.tile_pool(name="bigp", bufs=2) as bigp,
        tc.tile_pool(name="ps_pool", bufs=1, space="PSUM") as ps_pool,
    ):
        # ---- persistent tiles ----
        ident = pers.tile([P, P], BF16, name="ident")
        make_identity(nc, ident)
        eps_sb = pers.tile([P, 1], F32, name="eps_sb")
        nc.vector.memset(eps_sb, LN_EPS)
        ln16_sb = pers.tile([P, 1], F32, name="ln16_sb")
        nc.vector.memset(ln16_sb, float(np.log(REC_SCALE)))

        wq_sb = pers.tile([P, DC, D], BF16, name="wq_sb")
        nc.gpsimd.dma_start(out=wq_sb, in_=wq.rearrange("(c p) f -> p c f", p=P))
        wkv_sb = pers.tile([P, DC, 2 * D], BF16, name="wkv_sb")
        nc.gpsimd.dma_start(out=wkv_sb, in_=wkv.rearrange("(c p) f -> p c f", p=P))
        wout16_sb = pers.tile([P, H // 2, D], FP16, name="wout16_sb")
        nc.gpsimd.dma_start(out=wout16_sb, in_=wout16[:, :, :])

        xqT = bigp.tile([P, DC, N], BF16, tag="big", name="xqT")
        xkvT = bigp.tile([P, DC, N], BF16, tag="big", name="xkvT")
        qT = pers.tile([P, DC, N], BF16, name="qT")   # head 2c rows 0-63,
        kT = pers.tile([P, DC, N], BF16, name="kT")   # head 2c+1 rows 64-127
        # vaug fp16 [p, jt, head, col]: even heads v@0-63/ones@64-127,
        # odd heads ones@0-63/v@64-127
        vaug = pers.tile([P, NT, H, P], FP16, name="vaug")
        nc.vector.memset(vaug[:, :, 0::2, HD:P], 1.0)
        nc.vector.memset(vaug[:, :, 1::2, 0:HD], 1.0)
        # normalized attention out fp16: head 2hp rows 0-63, 2hp+1 rows 64-127
        aout16 = pers.tile([P, H // 2, N], FP16, name="aout16")

        # ---- phase A: layernorm + transpose (both streams) ----
        _phase_a(tc, lnx, lns, lnxs, ps_pool, xq, xqT, ident, eps_sb)
        _phase_a(tc, lnx, lns, lnxs, ps_pool, xkv, xkvT, ident, eps_sb)

        # ---- phase B: projections (bf16), v first ----
        # v token-major -> vaug fp16 (parity-split destinations)
        for mt in range(NT):
            ps = ps_pool.tile([P, D], F32, tag="A", bufs=2, name="psv")
            for k in range(DC):
                nc.tensor.matmul(
                    ps,
                    lhsT=xkvT[:, k, mt * P:(mt + 1) * P],
                    rhs=wkv_sb[:, k, D:2 * D],
                    start=(k == 0), stop=(k == DC - 1),
                )
            psr = ps.rearrange("p (h d) -> p h d", h=H)
            nc.scalar.copy(out=vaug[:, mt, 0::2, 0:HD], in_=psr[:, 0::2, :])
            nc.scalar.copy(out=vaug[:, mt, 1::2, HD:P], in_=psr[:, 1::2, :])

        for dst, w_sb, xT in ((qT, wq_sb, xqT), (kT, wkv_sb, xkvT)):
            for m in range(DC):
                for nch in range(4):
                    ps = ps_pool.tile([P, IC], F32, tag="A", bufs=2,
                                      name="ps")
                    for k in range(DC):
                        nc.tensor.matmul(
                            ps,
                            lhsT=w_sb[:, k, m * P:(m + 1) * P],
                            rhs=xT[:, k, nch * IC:(nch + 1) * IC],
                            start=(k == 0), stop=(k == DC - 1),
                        )
                    nc.vector.tensor_copy(
                        out=dst[:, m, nch * IC:(nch + 1) * IC], in_=ps
                    )
        # ---- phase C: attention ----
        # unit = (head pair hp, i-chunk iq). PSUM: simA/simB [128, 2, 512]
        # (2 banks each, jt-pair slots) + avA/avB [128, 512] (bufs=2 ring).
        # exp mix per unit: S10/D6 (even units) S9/D7 (odd).
        EXP_EVEN = [("s", "d"), ("d", "s")] * 4
        EXP_ODD = [("d", "s"), ("s", "d")] * 4

        def emit_exp(eng, ex, sim):
            sim_flat = sim.rearrange("p a b -> p (a b)")
            if eng == "s":
                nc.scalar.activation(out=ex.rearrange("p a b -> p (a b)"),
                                     in_=sim_flat, func=ACTF.Exp)
            else:
                nc.vector.tensor_scalar(
                    out=ex.rearrange("p a b -> p (a b)").bitcast(I16),
                    in0=sim_flat, scalar1=float(S16_A), scalar2=float(S16_B),
                    op0=ALU.mult, op1=ALU.add)

        norm_q = []
        for iq in range(4):
            for hp in range(H // 2):
                u = iq * 4 + hp
                avs = [
                    ps_pool.tile([P, IC], F32, tag=("A" if s == 0 else "av1"),
                                 bufs=2, name=f"av{s}")
                    for s in range(2)
                ]
                for jt in range(NT):
                    if True:
                        sim = ps_pool.tile([P, 2, IC], F32, tag="SIM",
                                           bufs=2, name="sim")
                        for s in range(2):
                            hb = s * HD
                            nc.tensor.matmul(
                                sim[:, s, :],
                                lhsT=kT[hb:hb + HD, hp,
                                        jt * P:(jt + 1) * P],
                                rhs=qT[hb:hb + HD, hp,
                                       iq * IC:(iq + 1) * IC],
                                start=True, stop=True,
                            )
                        ex = expp.tile([P, 2, IC], FP16, tag="ex", bufs=5,
                                       name="ex")
                        emit_exp(("s", "d")[(jt + u) % 2], ex, sim)
                        pend.append((avs, hp, jt, ex))
                        if jt in (6, 12) and norm_q:
                            norm_q.pop(0)()
                        if jt == 11 and hp >= H // 2 - 2 and phd_q:
                            phd_q.pop(0)()
                    if len(pend) > 4:
                        pavs, php, pjt, pex = pend.pop(0)
                        for s in range(2):
                            nc.tensor.matmul(
                                pavs[s],
                                lhsT=vaug[:, pjt, 2 * php + s, :],
                                rhs=pex[:, s, :],
                                start=(pjt == 0), stop=(pjt == NT - 1),
                            )
                def make_norm(avs, hp, iq):
                    def norm():
                        for s in range(2):
                            dlo = (1 - s) * HD    # D rows base
                            nlo = s * HD          # numerator rows base
                            lnD = normp.tile([P, IC], F32, tag=f"lnD{s}",
                                             name="lnD")
                            nc.scalar.activation(
                                out=lnD[dlo:dlo + HD, :],
                                in_=avs[s][dlo:dlo + HD, :],
                                func=ACTF.Ln)
                            recE = normp.tile([P, IC], F32, tag=f"recE{s}",
                                              name="recE")
                            nc.scalar.activation(
                                out=recE[dlo:dlo + HD, :],
                                in_=lnD[dlo:dlo + HD, :],
                                func=ACTF.Exp, scale=-1.0,
                                bias=ln16_sb[dlo:dlo + HD, :])
                            recN = normp.tile([P, IC], F32, tag=f"recN{s}",
                                              name="recN")
                            nc.sync.dma_start(out=recN[nlo:nlo + HD, :],
                                              in_=recE[dlo:dlo + HD, :])
                            nc.vector.tensor_tensor(
                                out=aout16[nlo:nlo + HD, hp,
                                           iq * IC:(iq + 1) * IC],
                                in0=avs[s][nlo:nlo + HD, :],
                                in1=recN[nlo:nlo + HD, :], op=ALU.mult)
                    return norm
                norm_q.append(make_norm(avs, hp, iq))

            # phase D for this iq's token tiles (i-chunk iq == token tiles
            # 4*iq..4*iq+3): runs on PE while the next iq's units start.
            while norm_q:
                norm_q.pop(0)()
            os_big = bigp.tile([P, 4, D], F32, tag="big", name="os_big")
            for j in range(4):
                it = iq * 4 + j
                pso = ps_pool.tile([P, D], F32, tag="av1", bufs=2,
                                   name="pso")
                for hp2 in range(H // 2):
                    nc.tensor.matmul(
                        pso,
                        lhsT=aout16[:, hp2, it * P:(it + 1) * P],
                        rhs=wout16_sb[:, hp2, :],
                        start=(hp2 == 0), stop=(hp2 == H // 2 - 1),
                    )
                nc.vector.tensor_scalar(
                    out=os_big[:, j, :], in0=pso,
                    scalar1=float(OUT_DESCALE), scalar2=0.0,
                    op0=ALU.mult, op1=ALU.add)
            nc.sync.dma_start(out=outs[iq][:, :, :], in_=os_big)

        # ---- phase D residual: nothing left (emitted per-iq above) ----


# revision 31
# speedup vs baseline: 1.5255x; 1.0113x over previous
"""Trainium2 Bass kernel: dual cross-attention block (nn_CA_36670430773307).

Full-input contract: kernel(**inputs) takes the complete unsharded tensors and
returns the complete (4, 4096, 512) output.

Sharding: 8 cores = batch(4) x direction(2). Each core computes one full
cross-attention direction (t->i or i->t) for one batch element.

v3 design (HW-measured microbenchmarks behind every choice):
  - sim (q@kT): bf16, K=64 head pairs row-grouped at partitions 0-63/64-127;
    a pair issues every ~260ns (true PE row-group concurrency).
  - exp: split Scalar (native Exp, PSUM->fp16, ~1.07ns/col) and DVE
    (Schraudolph-to-fp16 bits: int16 = round(1024/ln2*x + 15*1024-44),
    bit-reinterpreted as fp16; round-to-nearest conversion confirmed).
    GPSIMD cannot read PSUM so it only triggers DMAs.
  - av (attn@v): fp16, K=128 per j-tile. vaug [128, 128] per (jt, head):
    even heads: cols 0-63 = v, 64-127 = ones; odd heads flipped. The av
    output then carries the softmax denominator replicated on the OPPOSITE
    64 partitions from the numerator, and numerators of a head pair land on
    complementary partition halves -> out-projection runs K=128.
  - softmax 1/D: Ln then Exp(-x+ln16) on Scalar (4e-5 rel err), computed on
    the D rows in place, one SBUF->SBUF DMA to shift to the numerator's
    partitions, one DVE multiply (PSUM x SBUF -> fp16 aout).
  - out projection: fp16, K=128 (head pair per matmul), x16 descale folded
    into the final PSUM->SBUF copy.
  - gamma/beta: gamma folded into the projection weights host-side (exact);
    beta is guaranteed zero by the problem spec (asserted).
Precision (numpy pipeline sim): rel err ~1.0e-2 vs the 2e-2 gate.
"""

import numpy as np
import ml_dtypes

import concourse.bass as bass
import concourse.mybir as mybir
import concourse.tile as tile
from concourse.bass_utils import run_bass_kernel_spmd
from concourse.masks import make_identity

N = 2048            # tokens per stream
D = 512             # model dim
H = 8               # heads
HD = 64             # head dim
P = 128             # SBUF partitions
NT = N // P         # 16 token tiles
DC = D // P         # 4 model-dim chunks
IC = 512            # i-chunk (PSUM bank free size fp32)
LN_EPS = 1e-5

F32 = mybir.dt.float32
BF16 = mybir.dt.bfloat16
FP16 = mybir.dt.float16
I16 = mybir.dt.int16
ALU = mybir.AluOpType
ACTF = mybir.ActivationFunctionType

S16_A = 1024.0 / np.log(2.0)   # schraudolph-to-fp16: bits = A*x + B
S16_B = 15.0 * 1024 - 44.0
REC_SCALE = 16.0               # folded into Exp(-lnD + ln REC_SCALE)
OUT_DESCALE = 1.0 / REC_SCALE

LAST_EXEC_NS = None
_NC_CACHE = None


def _legalize_waits(js):
    """Walrus encodes ONE sync wait per instruction; split extras onto
    EventSemaphore instructions on the same engine."""
    for f in js["functions"]:
        for b in f["blocks"]:
            out = []
            for ins in b["instructions"]:
                si = ins.get("sync_info") or {}
                ow = si.get("on_wait") or []
                if len(ow) > 1:
                    for k, w in enumerate(ow[:-1]):
                        out.append({
                            "debug": ins.get("debug"),
                            "engine": ins["engine"],
                            "ins": [], "outs": [],
                            "name": f"{ins['name']}_w{k}",
                            "opcode": "EventSemaphore",
                            "sync_info": {"on_update": [], "on_wait": [w]},
                        })
                    si = dict(si)
                    si["on_wait"] = [ow[-1]]
                    ins = dict(ins)
                    ins["sync_info"] = si
                out.append(ins)
            b["instructions"] = out


def _build_program():
    nc = bass.Bass()

    xq = nc.declare_dram_parameter("xq", [N, D], F32, isOutput=False)
    xkv = nc.declare_dram_parameter("xkv", [N, D], F32, isOutput=False)
    wq = nc.declare_dram_parameter("wq", [D, D], BF16, isOutput=False)
    wkv = nc.declare_dram_parameter("wkv", [D, 2 * D], BF16, isOutput=False)
    wout16 = nc.declare_dram_parameter("wout16", [P, H // 2, D], FP16,
                                       isOutput=False)
    outs = [
        nc.declare_dram_parameter(f"out{g}", [P, 4, D], F32, isOutput=True)
        for g in range(NT // 4)
    ]

    with tile.TileContext(nc) as tc:
        _body(tc, xq, xkv, wq, wkv, wout16, outs)

    import json
    js = json.loads(nc.to_json_bytes())
    _legalize_waits(js)
    legalized = json.dumps(js).encode()
    nc.to_json_bytes = lambda: legalized
    return nc


def _phase_a(tc, lnx, lns, lnxs, ps_pool, src, xT, ident, eps_sb):
    """LayerNorm one stream token-major (gamma/beta folded into weights
    host-side), PE-transpose into d-major xT."""
    nc = tc.nc
    xbig = lnx.tile([P, NT, D], BF16, tag="xbig", name="xbig")
    src_r = src.rearrange("(t p) d -> p t d", p=P)
    qq = NT // 4
    for q in range(4):
        nc.gpsimd.dma_start(out=xbig[:, q * qq:(q + 1) * qq, :],
                            in_=src_r[:, q * qq:(q + 1) * qq, :])
    for itg in range(NT // 4):
        # batched stats: one Sqrt + one reciprocal per 4 tiles (fewer
        # cross-engine latency hops in the LN chain)
        mvs = lns.tile([P, 4, 2], F32, tag="mvs", name="mvs")
        for kk in range(4):
            it = itg * 4 + kk
            st = lns.tile([P, 6], F32, tag="st", name="st")
            nc.vector.bn_stats(out=st, in_=xbig[:, it, :])
            nc.vector.bn_aggr(out=mvs[:, kk, :], in_=st)
        iv = lns.tile([P, 4], F32, tag="iv", name="iv")
        nc.scalar.activation(out=iv, in_=mvs[:, :, 1], func=ACTF.Sqrt,
                             bias=eps_sb)
        nc.vector.reciprocal(out=iv, in_=iv)
        xss = []
        for kk in range(4):
            it = itg * 4 + kk
            xs = lnxs.tile([P, D], BF16, name="xs")
            nc.vector.tensor_scalar(
                out=xs, in0=xbig[:, it, :],
                scalar1=mvs[:, kk, 0:1], scalar2=iv[:, kk:kk + 1],
                op0=ALU.subtract, op1=ALU.mult,
            )
            xss.append(xs)
        for c in range(DC):
            ps = ps_pool.tile([P, 4 * P], BF16, tag="A", bufs=2, name="tp")
            for kk in range(4):
                nc.tensor.transpose(
                    ps[:, kk * P:(kk + 1) * P],
                    xss[kk][:, c * P:(c + 1) * P],
                    ident,
                )
            nc.scalar.copy(
                out=xT[:, c, itg * 512:(itg + 1) * 512], in_=ps)


def _body(tc, xq, xkv, wq, wkv, wout16, outs):
    nc = tc.nc

    with (
        tc.tile_pool(name="persist", bufs=1) as pers,
        tc.tile_pool(name="lns", bufs=16) as lns,
        tc.tile_pool(name="lnxs", bufs=12) as lnxs,
        tc.tile_pool(name="lnx", bufs=2) as lnx,
        tc.tile_pool(name="expp", bufs=3) as expp,
        tc.tile_pool(name="normp", bufs=2) as normp,
        tc.tile_pool(name="bigp", bufs=2) as bigp,
        tc.tile_pool(name="ps_pool", bufs=1, space="PSUM") as ps_pool,
    ):
        # ---- persistent tiles ----
        ident = pers.tile([P, P], BF16, name="ident")
        make_identity(nc, ident)
        eps_sb = pers.tile([P, 1], F32, name="eps_sb")
        nc.vector.memset(eps_sb, LN_EPS)
        ln16_sb = pers.tile([P, 1], F32, name="ln16_sb")
        nc.vector.memset(ln16_sb, float(np.log(REC_SCALE)))

        wq_sb = pers.tile([P, DC, D], BF16, name="wq_sb")
        nc.gpsimd.dma_start(out=wq_sb, in_=wq.rearrange("(c p) f -> p c f", p=P))
        wkv_sb = pers.tile([P, DC, 2 * D], BF16, name="wkv_sb")
        nc.gpsimd.dma_start(out=wkv_sb, in_=wkv.rearrange("(c p) f -> p c f", p=P))
        wout16_sb = pers.tile([P, H // 2, D], FP16, name="wout16_sb")
        nc.gpsimd.dma_start(out=wout16_sb, in_=wout16[:, :, :])

        xqT = bigp.tile([P, DC, N], BF16, tag="big", name="xqT")
        xkvT = bigp.tile([P, DC, N], BF16, tag="big", name="xkvT")
        qT = pers.tile([P, DC, N], BF16, name="qT")   # head 2c rows 0-63,
        kT = pers.tile([P, DC, N], BF16, name="kT")   # head 2c+1 rows 64-127
        # vaug fp16 [p, jt, head, col]: even heads v@0-63/ones@64-127,
        # odd heads ones@0-63/v@64-127
        vaug = pers.tile([P, NT, H, P], FP16, name="vaug")
        nc.vector.memset(vaug[:, :, 0::2, HD:P], 1.0)
        nc.vector.memset(vaug[:, :, 1::2, 0:HD], 1.0)
        # normalized attention out fp16: head 2hp rows 0-63, 2hp+1 rows 64-127
        aout16 = pers.tile([P, H // 2, N], FP16, name="aout16")

        # ---- phase A: layernorm + transpose (both streams) ----
        _phase_a(tc, lnx, lns, lnxs, ps_pool, xq, xqT, ident, eps_sb)
        _phase_a(tc, lnx, lns, lnxs, ps_pool, xkv, xkvT, ident, eps_sb)

        # ---- phase B: projections (bf16), v first ----
        # v token-major -> vaug fp16 (parity-split destinations)
        for mt in range(NT):
            ps = ps_pool.tile([P, D], F32, tag="A", bufs=2, name="psv")
            for k in range(DC):
                nc.tensor.matmul(
                    ps,
                    lhsT=xkvT[:, k, mt * P:(mt + 1) * P],
                    rhs=wkv_sb[:, k, D:2 * D],
                    start=(k == 0), stop=(k == DC - 1),
                )
            psr = ps.rearrange("p (h d) -> p h d", h=H)
            nc.scalar.copy(out=vaug[:, mt, 0::2, 0:HD], in_=psr[:, 0::2, :])
            nc.scalar.copy(out=vaug[:, mt, 1::2, HD:P], in_=psr[:, 1::2, :])

        for dst, w_sb, xT in ((qT, wq_sb, xqT), (kT, wkv_sb, xkvT)):
            for m in range(DC):
                for nch in range(4):
                    ps = ps_pool.tile([P, IC], F32, tag="A", bufs=2,
                                      name="ps")
                    for k in range(DC):
                        nc.tensor.matmul(
                            ps,
                            lhsT=w_sb[:, k, m * P:(m + 1) * P],
                            rhs=xT[:, k, nch * IC:(nch + 1) * IC],
                            start=(k == 0), stop=(k == DC - 1),
                        )
                    nc.vector.tensor_copy(
                        out=dst[:, m, nch * IC:(nch + 1) * IC], in_=ps
                    )
        # ---- phase C: attention ----
        # unit = (head pair hp, i-chunk iq). PSUM: simA/simB [128, 2, 512]
        # (2 banks each, jt-pair slots) + avA/avB [128, 512] (bufs=2 ring).
        # exp mix per unit: S10/D6 (even units) S9/D7 (odd).
        EXP_EVEN = [("s", "d"), ("d", "s")] * 4
        EXP_ODD = [("d", "s"), ("s", "d")] * 4

        def emit_exp(eng, ex, sim):
            sim_flat = sim.rearrange("p a b -> p (a b)")
            if eng == "s":
                nc.scalar.activation(out=ex.rearrange("p a b -> p (a b)"),
                                     in_=sim_flat, func=ACTF.Exp)
            else:
                nc.vector.tensor_scalar(
                    out=ex.rearrange("p a b -> p (a b)").bitcast(I16),
                    in0=sim_flat, scalar1=float(S16_A), scalar2=float(S16_B),
                    op0=ALU.mult, op1=ALU.add)

        norm_q = []
        for iq in range(4):
            for hp in range(H // 2):
                u = iq * 4 + hp
                avs = [
                    ps_pool.tile([P, IC], F32, tag=("A" if s == 0 else "av1"),
                                 bufs=2, name=f"av{s}")
                    for s in range(2)
                ]
                for jt in range(NT):
                    if True:
                        sim = ps_pool.tile([P, 2, IC], F32, tag="SIM",
                                           bufs=2, name="sim")
                        for s in range(2):
                            hb = s * HD
                            nc.tensor.matmul(
                                sim[:, s, :],
                                lhsT=kT[hb:hb + HD, hp,
                                        jt * P:(jt + 1) * P],
                                rhs=qT[hb:hb + HD, hp,
                                       iq * IC:(iq + 1) * IC],
                                start=True, stop=True,
                            )
                        ex = expp.tile([P, 2, IC], FP16, tag="ex", bufs=5,
                                       name="ex")
                        emit_exp(("s", "d")[(jt + u) % 2], ex, sim)
                        pend.append((avs, hp, jt, ex))
                        if jt in (6, 12) and norm_q:
                            norm_q.pop(0)()
                        if jt == 11 and hp >= H // 2 - 2 and phd_q:
                            phd_q.pop(0)()
                    if len(pend) > (1 if u == 15 and jt >= 12 else 4):
                        pavs, php, pjt, pex = pend.pop(0)
                        for s in range(2):
                            nc.tensor.matmul(
                                pavs[s],
                                lhsT=vaug[:, pjt, 2 * php + s, :],
                                rhs=pex[:, s, :],
                                start=(pjt == 0), stop=(pjt == NT - 1),
                            )
                def make_norm(avs, hp, iq):
                    def norm():
                        for s in range(2):
                            dlo = (1 - s) * HD    # D rows base
                            nlo = s * HD          # numerator rows base
                            lnD = normp.tile([P, IC], F32, tag=f"lnD{s}",
                                             name="lnD")
                            nc.scalar.activation(
                                out=lnD[dlo:dlo + HD, :],
                                in_=avs[s][dlo:dlo + HD, :],
                                func=ACTF.Ln)
                            recE = normp.tile([P, IC], F32, tag=f"recE{s}",
                                              name="recE")
                            nc.scalar.activation(
                                out=recE[dlo:dlo + HD, :],
                                in_=lnD[dlo:dlo + HD, :],
                                func=ACTF.Exp, scale=-1.0,
                                bias=ln16_sb[dlo:dlo + HD, :])
                            recN = normp.tile([P, IC], F32, tag=f"recN{s}",
                                              name="recN")
                            nc.sync.dma_start(out=recN[nlo:nlo + HD, :],
                                              in_=recE[dlo:dlo + HD, :])
                            nc.vector.tensor_tensor(
                                out=aout16[nlo:nlo + HD, hp,
                                           iq * IC:(iq + 1) * IC],
                                in0=avs[s][nlo:nlo + HD, :],
                                in1=recN[nlo:nlo + HD, :], op=ALU.mult)
                    return norm
                norm_q.append(make_norm(avs, hp, iq))

            # phase D for this iq's token tiles (i-chunk iq == token tiles
            # 4*iq..4*iq+3): runs on PE while the next iq's units start.
            while norm_q:
                norm_q.pop(0)()
            os_big = bigp.tile([P, 4, D], F32, tag="big", name="os_big")
            for j in range(4):
                it = iq * 4 + j
                pso = ps_pool.tile([P, D], F32, tag="av1", bufs=2,
                                   name="pso")
                for hp2 in range(H // 2):
                    nc.tensor.matmul(
                        pso,
                        lhsT=aout16[:, hp2, it * P:(it + 1) * P],
                        rhs=wout16_sb[:, hp2, :],
                        start=(hp2 == 0), stop=(hp2 == H // 2 - 1),
                    )
                nc.vector.tensor_scalar(
                    out=os_big[:, j, :], in0=pso,
                    scalar1=float(OUT_DESCALE), scalar2=0.0,
                    op0=ALU.mult, op1=ALU.add)
            nc.sync.dma_start(out=outs[iq][:, :, :], in_=os_big)

        # ---- phase D residual: nothing left (emitted per-iq above) ----
